# revision 4
# baseline (speedup 1.0000x reference)
"""Trainium2 Bass kernel for nn_DeformBasicBlock1 (deformable conv block).

Single SPMD invocation over 8 cores, sharded by channel-group g (8
x-channels / 81 offset channels / 8 output channels per core).  All
cross-core exchange happens on-device: AllGather replicates the padded
activation volumes for the offset convs, ReduceScatter sums the per-group
deform-conv partials so each core lands exactly its own 8 output channels
for the per-channel BN.  Host traffic per call is just the sharded inputs
(~23 MB) and the sharded output (~13 MB); the compiled executable is cached
across calls.

Deform sampling follows the proven dense 5x5x5 shifted-hat expansion
(offsets clamped to +/-1.999), contracted with the deform weights on the PE.
"""
import json
import os
import numpy as np

os.environ.setdefault("JAX_COMPILATION_CACHE_DIR", "/tmp/jax_cache_deform")
os.environ.setdefault("JAX_PERSISTENT_CACHE_MIN_COMPILE_TIME_SECS", "1")

import concourse.bass as bass
import concourse.mybir as mybir
from concourse.tile import TileContext
import concourse.bass_utils as bass_utils
import concourse.tile_utils as tile_utils

# ---------------------------------------------------------------- tilefix --
# This container's walrus rejects >1 sem-wait per instruction; split extra
# waits onto preceding same-engine NoOps (program order preserves wait
# semantics).
_orig_compile_bir_kernel = bass_utils.compile_bir_kernel


def _split_waits_json(bir_json: bytes) -> bytes:
    j = json.loads(bir_json)
    ctr = 0
    changed = False
    for f in j["functions"]:
        for b in f["blocks"]:
            insts = b["instructions"]
            if not any(
                len((i.get("sync_info") or {}).get("on_wait") or []) > 1
                for i in insts
            ):
                continue
            changed = True
            out = []
            for inst in insts:
                si = inst.get("sync_info")
                if si:
                    ow = si.get("on_wait") or []
                    if len(ow) > 1:
                        for w in ow[:-1]:
                            ctr += 1
                            nop = {
                                "engine": inst["engine"],
                                "ins": [],
                                "outs": [],
                                "name": f"WSPLIT-{ctr}",
                                "opcode": "NoOp",
                                "sync_info": {"on_update": [], "on_wait": [w]},
                            }
                            if "debug" in inst:
                                nop["debug"] = inst["debug"]
                            out.append(nop)
                        si["on_wait"] = [ow[-1]]
                out.append(inst)
            b["instructions"] = out
    return json.dumps(j).encode() if changed else bir_json


def _patched_compile_bir_kernel(bir_json, tmpdir, neff_name="file.neff"):
    if isinstance(bir_json, str):
        bir_json = bir_json.encode()
    return _orig_compile_bir_kernel(_split_waits_json(bir_json), tmpdir, neff_name)


bass_utils.compile_bir_kernel = _patched_compile_bir_kernel
import concourse.bass2jax as _b2j  # noqa: E402

_b2j.compile_bir_kernel = _patched_compile_bir_kernel
try:
    tile_utils.max_sbuf_usage = 204 * 1024
except Exception:
    pass

# ------------------------------------------------------------- constants --
NCORES = 8
B, D, H, W = 2, 8, 56, 56
CPG, G, K = 8, 8, 27
OCG = 81
V = D * H * W
BV = B * V
PLANE = 3364  # 58*58
NB, BH = 14, 4
P = NB * CPG  # 112
CH = D * BH * W  # 1792
XD, XH, XWW = 14, 10, 62
XSZ = XD * XH * XWW
XVOL = XD * 62 * 62
XPL = 62 * 62  # 3844
SS = 5
CLAMP = 1.999
F32 = mybir.dt.float32
BF16 = mybir.dt.bfloat16
AX = mybir.AxisListType
ALU = mybir.AluOpType
ACTF = mybir.ActivationFunctionType
RG = [list(range(NCORES))]


def mkap(tile, off, dims):
    ap = tile[:]
    return bass.AP(tensor=ap.tensor, offset=ap.offset + off,
                   ap=[list(ap.ap[0])] + [list(d) for d in dims])


def dmkap(t_ap, off, dims):
    return bass.AP(tensor=t_ap.tensor, offset=t_ap.offset + off,
                   ap=[list(d) for d in dims])


ZCHUNK = 4096


def zero_dram(nc, zero_sb, dram_ap, nparts, total):
    for off in range(0, total, ZCHUNK):
        sz = min(ZCHUNK, total - off)
        nc.sync.dma_start(
            out=dmkap(dram_ap, off, [[total, nparts], [1, sz]]),
            in_=dmkap(zero_sb[:], 0, [[ZCHUNK, nparts], [1, sz]]))


def conv_phase(nc, tc, xpad_dram, wt_sb, bias_sb, off_dram, sfx):
    """27-tap conv: xpad_dram [64, B*10*PLANE] -> off_dram [81, B*D*3136]."""
    GUARD = 64
    # row-aligned n-chunks of the padded plane: (n0, nsz, valid_row_start, n_valid)
    CHUNKS = []
    for r0 in range(0, 58, 8):
        nr = min(8, 58 - r0)
        v0 = max(1, r0)
        v1 = min(57, r0 + nr)
        CHUNKS.append((r0 * 58, nr * 58, v0 - r0, v1 - v0))
    with tc.tile_pool(name=f"convp{sfx}", bufs=2) as pool, \
         tc.tile_pool(name=f"convps{sfx}", bufs=4, space="PSUM") as pspool:
        for b in range(B):
            for j in range(4):
                xpc = pool.tile([64, 2 * GUARD + 4 * PLANE], F32, tag="xpc")
                nc.vector.memset(xpc[:, :GUARD], 0.0)
                nc.vector.memset(xpc[:, GUARD + 4 * PLANE:], 0.0)
                nc.sync.dma_start(
                    out=xpc[:, GUARD:GUARD + 4 * PLANE],
                    in_=dmkap(xpad_dram[:], (b * 10 + 2 * j) * PLANE,
                              [[B * 10 * PLANE, 64], [1, 4 * PLANE]]))
                for ds in range(2):
                    d = 2 * j + ds
                    for (n0, nsz, vr, nv) in CHUNKS:
                        ps = pspool.tile([OCG, 512], F32, tag="cps")
                        for k in range(K):
                            kd, kh, kw = k // 9, (k // 3) % 3, k % 3
                            roff = GUARD + (ds + kd) * PLANE + (kh - 1) * 58 + (kw - 1) + n0
                            nc.tensor.matmul(ps[:, :nsz], wt_sb[:, k, :],
                                             mkap(xpc, roff, [[1, nsz]]),
                                             start=(k == 0), stop=(k == K - 1))
                        ot = pool.tile([OCG, 512], F32, tag="convot")
                        nc.vector.tensor_tensor(
                            out=ot[:, :nsz], in0=ps[:, :nsz],
                            in1=mkap(bias_sb, 0, [[0, nsz]]), op=ALU.add)
                        if nv <= 0:
                            continue
                        # store valid interior rows to canonical [81, B, D, 56, 56]
                        real_r0 = n0 // 58 + vr - 1
                        nc.sync.dma_start(
                            out=dmkap(off_dram[:], (b * D + d) * 3136 + real_r0 * 56,
                                      [[B * D * 3136, OCG], [1, nv * 56]]),
                            in_=mkap(ot, vr * 58 + 1, [[58, nv], [1, 56]]))


def dense_phase(nc, tc, xw_dram, off_dram, wd_sb, partial_dram, colsd_dram, sfx):
    """Dense 5^3 deform + einsum -> partial_dram [64, B*V] (band-perm)."""
    with tc.tile_pool(name=f"densep{sfx}", bufs=1) as pool, \
         tc.tile_pool(name=f"densew{sfx}", bufs=1) as wpool, \
         tc.tile_pool(name=f"denseps{sfx}", bufs=2, space="PSUM") as pspool:
        for b in range(B):
            xw = pool.tile([P, XSZ], F32, tag="xw")
            for dd in range(XD):
                nc.sync.dma_start(
                    out=mkap(xw, dd * XH * XWW, [[1, 620]]),
                    in_=dmkap(xw_dram[:], b * XVOL + dd * 62 * 62,
                              [[BH * XWW, NB], [B * XVOL, CPG], [1, XH * XWW]]))
            for k in range(K):
                kd, kh, kw = k // 9 - 1, (k // 3) % 3 - 1, k % 3 - 1
                offt = pool.tile([P, 3, CH], F32, tag="offt")
                for ax in range(3):
                    for dd in range(D):
                        nc.sync.dma_start(
                            out=mkap(offt, ax * CH + dd * BH * W, [[1, BH * W]]),
                            in_=dmkap(off_dram[:],
                                      (3 * k + ax) * B * D * 3136 + (b * D + dd) * 3136,
                                      [[BH * W, NB], [0, CPG], [1, BH * W]]))
                nc.vector.tensor_scalar(out=offt[:], in0=offt[:], scalar1=CLAMP,
                                        scalar2=-CLAMP, op0=ALU.min, op1=ALU.max)
                hw = pool.tile([P, SS, CH], F32, tag="hw")
                for a in range(SS):
                    nc.scalar.activation(hw[:, a, :], offt[:, 2, :], ACTF.Abs,
                                         bias=float(-(a - 2)), scale=1.0)
                    nc.scalar.activation(hw[:, a, :], hw[:, a, :], ACTF.Relu,
                                         bias=1.0, scale=-1.0)
                cols = wpool.tile([P, CH], F32, tag="cols")
                pt = wpool.tile([P, CH], F32, tag="pt")
                at = wpool.tile([P, CH], F32, tag="at")
                tt = wpool.tile([P, CH], F32, tag="tt")
                hdsl = pool.tile([P, CH], F32, tag="hdsl")
                hhsl = pool.tile([P, CH], F32, tag="hhsl")
                first = True
                for sd in range(SS):
                    nc.scalar.activation(hdsl[:], offt[:, 0, :], ACTF.Abs,
                                         bias=float(-(sd - 2)), scale=1.0)
                    nc.scalar.activation(hdsl[:], hdsl[:], ACTF.Relu,
                                         bias=1.0, scale=-1.0)
                    for sh in range(SS):
                        nc.scalar.activation(hhsl[:], offt[:, 1, :], ACTF.Abs,
                                             bias=float(-(sh - 2)), scale=1.0)
                        nc.scalar.activation(hhsl[:], hhsl[:], ACTF.Relu,
                                             bias=1.0, scale=-1.0)
                        nc.vector.tensor_tensor(out=pt[:], in0=hdsl[:],
                                                in1=hhsl[:], op=ALU.mult)
                        for sw in range(SS):
                            xoff = ((1 + kd + sd) * XH * XWW + (1 + kh + sh) * XWW
                                    + (1 + kw + sw))
                            xap = mkap(xw, xoff, [[XH * XWW, D], [XWW, BH], [1, W]])
                            dst = at if sw == 0 else tt
                            nc.vector.tensor_tensor(out=dst[:], in0=xap,
                                                    in1=hw[:, sw, :], op=ALU.mult)
                            if sw > 0:
                                nc.vector.tensor_tensor(out=at[:], in0=at[:],
                                                        in1=tt[:], op=ALU.add)
                        if first:
                            nc.vector.tensor_tensor(out=cols[:], in0=pt[:], in1=at[:],
                                                    op=ALU.mult)
                            first = False
                        else:
                            nc.gpsimd.tensor_tensor(out=tt[:], in0=pt[:], in1=at[:],
                                                    op=ALU.mult)
                            nc.gpsimd.tensor_tensor(out=cols[:], in0=cols[:], in1=tt[:],
                                                    op=ALU.add)
                nc.sync.dma_start(
                    out=dmkap(colsd_dram[:], (b * K + k) * CH,
                              [[B * K * CH, P], [1, CH]]),
                    in_=cols[:])
            tc.strict_bb_all_engine_barrier()
            # einsum: psum accumulate over taps per band
            for hb in range(NB):
                ps2 = pspool.tile([64, 2048], F32, tag="eps")
                for k in range(K):
                    cr = wpool.tile([CPG, CH], F32, tag="colsr")
                    nc.sync.dma_start(
                        out=cr[:],
                        in_=dmkap(colsd_dram[:], hb * CPG * B * K * CH + (b * K + k) * CH,
                                  [[B * K * CH, CPG], [1, CH]]))
                    for i in range(4):
                        nc.tensor.matmul(ps2[:, i * 512:i * 512 + 448], wd_sb[:, k, :],
                                         cr[:, i * 448:(i + 1) * 448],
                                         start=(k == 0), stop=(k == K - 1))
                pot = wpool.tile([64, CH], F32, tag="pot")
                nc.vector.tensor_copy(out=pot[:], in_=mkap(ps2, 0, [[512, 4], [1, 448]]))
                nc.sync.dma_start(
                    out=dmkap(partial_dram[:], b * V + hb * CH, [[B * V, 64], [1, CH]]),
                    in_=pot[:])


def ensure_consts(nc):
    for v in (2.0, -2.0, -1.0, 1e-5):
        key = (F32, v)
        if key not in nc.const_aps.aps:
            t = nc.alloc_sbuf_tensor(f"const-f32-{v}", [128, 1], F32)
            nc.gpsimd.memset(t.ap(), v)
            nc.const_aps.aps[key] = t.ap()


def bn_stats_g(nc, pool, spool, src_dram, gamma_ap, beta_ap, sfx):
    """Per-channel BN scale/shift for the CPG-channel shard src [CPG, B*V]."""
    NCHK = 16
    CSZ = BV // NCHK
    sum_t = spool.tile([CPG, 1], F32, tag=f"bsum{sfx}")
    sq_t = spool.tile([CPG, 1], F32, tag=f"bsq{sfx}")
    nc.vector.memset(sum_t[:], 0.0)
    nc.vector.memset(sq_t[:], 0.0)
    for i in range(NCHK):
        ht = pool.tile([CPG, CSZ], F32, tag="bst")
        sqv = pool.tile([CPG, CSZ], F32, tag="bsv")
        t1 = pool.tile([CPG, 1], F32, tag="bt1")
        t2 = pool.tile([CPG, 1], F32, tag="bt2")
        nc.sync.dma_start(out=ht[:],
                          in_=dmkap(src_dram[:], i * CSZ, [[BV, CPG], [1, CSZ]]))
        nc.vector.tensor_reduce(out=t1[:], in_=ht[:], axis=AX.X, op=ALU.add)
        nc.vector.tensor_tensor(out=sqv[:], in0=ht[:], in1=ht[:], op=ALU.mult)
        nc.vector.tensor_reduce(out=t2[:], in_=sqv[:], axis=AX.X, op=ALU.add)
        nc.vector.tensor_tensor(out=sum_t[:], in0=sum_t[:], in1=t1[:], op=ALU.add)
        nc.vector.tensor_tensor(out=sq_t[:], in0=sq_t[:], in1=t2[:], op=ALU.add)
    N = float(BV)
    scale = spool.tile([CPG, 1], F32, tag=f"bscale{sfx}")
    shift = spool.tile([CPG, 1], F32, tag=f"bshift{sfx}")
    mean = spool.tile([CPG, 1], F32, tag=f"bmean{sfx}")
    var = spool.tile([CPG, 1], F32, tag=f"bvar{sfx}")
    msq = spool.tile([CPG, 1], F32, tag=f"bmsq{sfx}")
    rstd = spool.tile([CPG, 1], F32, tag=f"brstd{sfx}")
    nc.vector.tensor_scalar(out=mean[:], in0=sum_t[:], scalar1=1.0 / N, scalar2=0.0,
                            op0=ALU.mult, op1=ALU.add)
    nc.vector.tensor_scalar(out=var[:], in0=sq_t[:], scalar1=1.0 / N, scalar2=0.0,
                            op0=ALU.mult, op1=ALU.add)
    nc.vector.tensor_tensor(out=msq[:], in0=mean[:], in1=mean[:], op=ALU.mult)
    nc.vector.tensor_tensor(out=var[:], in0=var[:], in1=msq[:], op=ALU.subtract)
    nc.scalar.activation(out=rstd[:], in_=var[:], func=ACTF.Sqrt, bias=1e-5, scale=1.0)
    nc.vector.reciprocal(out=rstd[:], in_=rstd[:])
    nc.vector.tensor_tensor(out=scale[:], in0=gamma_ap, in1=rstd[:], op=ALU.mult)
    nc.vector.tensor_tensor(out=shift[:], in0=mean[:], in1=scale[:], op=ALU.mult)
    nc.vector.tensor_tensor(out=shift[:], in0=beta_ap, in1=shift[:], op=ALU.subtract)
    return scale, shift


def bn1_phase(nc, tc, hs_dram, gb_sb, hpad_loc, hw_loc):
    """BN+ReLU on the 8-channel shard (band-perm) and scatter into the padded
    DRAM layouts for layer 2."""
    with tc.tile_pool(name="bn1p", bufs=2) as pool, \
         tc.tile_pool(name="bn1s", bufs=1) as spool:
        scale, shift = bn_stats_g(nc, pool, spool, hs_dram,
                                  gb_sb[:, 0:1], gb_sb[:, 1:2], "1")
        for b in range(B):
            for hb in range(NB):
                ht = pool.tile([CPG, CH], F32, tag="bna")
                nc.sync.dma_start(
                    out=ht[:],
                    in_=dmkap(hs_dram[:], (b * NB + hb) * CH, [[BV, CPG], [1, CH]]))
                nc.scalar.activation(out=ht[:], in_=ht[:], func=ACTF.Relu,
                                     bias=shift[:], scale=scale[:])
                for d in range(D):
                    src = mkap(ht, d * BH * W, [[W, BH], [1, W]])
                    nc.sync.dma_start(
                        out=dmkap(hpad_loc[:],
                                  (b * 10 + d + 1) * PLANE + (hb * BH + 1) * 58 + 1,
                                  [[B * 10 * PLANE, CPG], [58, BH], [1, W]]),
                        in_=src)
                    src2 = mkap(ht, d * BH * W, [[W, BH], [1, W]])
                    nc.sync.dma_start(
                        out=dmkap(hw_loc[:],
                                  b * XVOL + (d + 3) * XPL + (hb * BH + 3) * 62 + 3,
                                  [[B * XVOL, CPG], [62, BH], [1, W]]),
                        in_=src2)


def bn2_phase(nc, tc, hs_dram, gb_sb, xs_in, out_par):
    """BN + residual + ReLU on the 8-channel shard; write canonical output."""
    with tc.tile_pool(name="bn2p", bufs=2) as pool, \
         tc.tile_pool(name="bn2s", bufs=1) as spool:
        scale, shift = bn_stats_g(nc, pool, spool, hs_dram,
                                  gb_sb[:, 0:1], gb_sb[:, 1:2], "2")
        for b in range(B):
            for hb in range(NB):
                ht = pool.tile([CPG, CH], F32, tag="b2h")
                rt = pool.tile([CPG, CH], F32, tag="b2r")
                nc.sync.dma_start(
                    out=ht[:],
                    in_=dmkap(hs_dram[:], (b * NB + hb) * CH, [[BV, CPG], [1, CH]]))
                nc.sync.dma_start(
                    out=rt[:],
                    in_=dmkap(xs_in[:], b * V + hb * BH * W,
                              [[BV, CPG], [H * W, D], [1, BH * W]]))
                nc.vector.tensor_tensor(out=ht[:], in0=ht[:],
                                        in1=mkap(scale, 0, [[0, CH]]), op=ALU.mult)
                nc.vector.tensor_tensor(out=ht[:], in0=ht[:],
                                        in1=mkap(shift, 0, [[0, CH]]), op=ALU.add)
                nc.vector.tensor_tensor(out=ht[:], in0=ht[:], in1=rt[:], op=ALU.add)
                ob = pool.tile([CPG, CH], BF16, tag="b2o")
                nc.vector.tensor_scalar(out=ob[:], in0=ht[:], scalar1=0.0,
                                        scalar2=0.0, op0=ALU.max, op1=ALU.add)
                nc.sync.dma_start(
                    out=dmkap(out_par[:], b * V + hb * BH * W,
                              [[BV, CPG], [H * W, D], [1, BH * W]]),
                    in_=ob[:])


DEBUG_TAPS = bool(os.environ.get("KERNEL_DEBUG_TAPS"))


def _build_program():
    nc = bass.Bass("TRN2", target_bir_lowering=False, num_devices=NCORES)
    ensure_consts(nc)
    xs_in = nc.declare_dram_parameter("xs", [CPG, BV], BF16, isOutput=False)
    wt1_in = nc.declare_dram_parameter("wt1", [64, K * OCG], BF16, isOutput=False)
    bo1_in = nc.declare_dram_parameter("bo1", [OCG, 1], F32, isOutput=False)
    wd1_in = nc.declare_dram_parameter("wd1", [CPG, K * 64], BF16, isOutput=False)
    wt2_in = nc.declare_dram_parameter("wt2", [64, K * OCG], BF16, isOutput=False)
    bo2_in = nc.declare_dram_parameter("bo2", [OCG, 1], F32, isOutput=False)
    wd2_in = nc.declare_dram_parameter("wd2", [CPG, K * 64], BF16, isOutput=False)
    gb1_in = nc.declare_dram_parameter("gb1", [CPG, 2], F32, isOutput=False)
    gb2_in = nc.declare_dram_parameter("gb2", [CPG, 2], F32, isOutput=False)
    out_par = nc.declare_dram_parameter("out", [CPG, BV], BF16, isOutput=True)

    xs32_d = nc.dram_tensor("xs32_d", [CPG, BV], F32)
    xpad_loc = nc.dram_tensor("xpad_loc", [CPG, B * 10 * PLANE], F32)
    xpad_full = nc.dram_tensor("xpad_full", [64, B * 10 * PLANE], F32)
    xw_loc = nc.dram_tensor("xw_loc", [CPG, B * XVOL], F32)
    off1_d = nc.dram_tensor("off1_d", [OCG, B * D * 3136], F32)
    cols1_d = nc.dram_tensor("cols1_d", [P, B * K * CH], F32)
    part1_d = nc.dram_tensor("part1_d", [64, BV], F32)
    h1s_d = nc.dram_tensor("h1s_d", [CPG, BV], F32)
    hpad_loc = nc.dram_tensor("hpad_loc", [CPG, B * 10 * PLANE], F32)
    hpad_full = nc.dram_tensor("hpad_full", [64, B * 10 * PLANE], F32)
    hw_loc = nc.dram_tensor("hw_loc", [CPG, B * XVOL], F32)
    off2_d = nc.dram_tensor("off2_d", [OCG, B * D * 3136], F32)
    cols2_d = nc.dram_tensor("cols2_d", [P, B * K * CH], F32)
    part2_d = nc.dram_tensor("part2_d", [64, BV], F32)
    h2s_d = nc.dram_tensor("h2s_d", [CPG, BV], F32)

    with TileContext(nc) as tc:
        with tc.tile_pool(name="persist", bufs=1) as sp:
            zero_sb = sp.tile([CPG, ZCHUNK], F32, tag="zsb")
            nc.vector.memset(zero_sb[:], 0.0)
            wt1_sb = sp.tile([64, K, OCG], F32, tag="wt1")
            bo1_sb = sp.tile([OCG, 1], F32, tag="bo1")
            nc.sync.dma_start(out=bo1_sb[:], in_=bo1_in[:])
            wd1_sb = sp.tile([CPG, K, 64], F32, tag="wd1")
            wt2_sb = sp.tile([64, K, OCG], F32, tag="wt2")
            bo2_sb = sp.tile([OCG, 1], F32, tag="bo2")
            nc.sync.dma_start(out=bo2_sb[:], in_=bo2_in[:])
            wd2_sb = sp.tile([CPG, K, 64], F32, tag="wd2")
            with tc.tile_pool(name="wload", bufs=1) as wl:
                for src, dst, parts, cols in ((wt1_in, wt1_sb, 64, K * OCG),
                                              (wt2_in, wt2_sb, 64, K * OCG),
                                              (wd1_in, wd1_sb, CPG, K * 64),
                                              (wd2_in, wd2_sb, CPG, K * 64)):
                    stg = wl.tile([parts, cols], BF16, tag=f"stg{parts}")
                    nc.sync.dma_start(out=stg[:], in_=src[:])
                    nc.vector.tensor_copy(out=dst[:], in_=stg[:])
            gb1_sb = sp.tile([CPG, 2], F32, tag="gb1")
            nc.sync.dma_start(out=gb1_sb[:], in_=gb1_in[:])
            gb2_sb = sp.tile([CPG, 2], F32, tag="gb2")
            nc.sync.dma_start(out=gb2_sb[:], in_=gb2_in[:])

            # zero the padded scratch volumes (borders stay zero throughout)
            zero_dram(nc, zero_sb, xpad_loc[:], CPG, B * 10 * PLANE)
            zero_dram(nc, zero_sb, xw_loc[:], CPG, B * XVOL)
            zero_dram(nc, zero_sb, hpad_loc[:], CPG, B * 10 * PLANE)
            zero_dram(nc, zero_sb, hw_loc[:], CPG, B * XVOL)
            # widen the bf16 input shard to f32
            with tc.tile_pool(name="cvt", bufs=2) as cpool:
                NCV = 16
                CVS = BV // NCV
                for i in range(NCV):
                    tb = cpool.tile([CPG, CVS], BF16, tag="cvb")
                    tf = cpool.tile([CPG, CVS], F32, tag="cvf")
                    nc.sync.dma_start(
                        out=tb[:],
                        in_=dmkap(xs_in[:], i * CVS, [[BV, CPG], [1, CVS]]))
                    nc.vector.tensor_copy(out=tf[:], in_=tb[:])
                    nc.sync.dma_start(
                        out=dmkap(xs32_d[:], i * CVS, [[BV, CPG], [1, CVS]]),
                        in_=tf[:])
            tc.strict_bb_all_engine_barrier()
            # scatter own x channels into the padded layouts
            for b in range(B):
                for d in range(D):
                    nc.sync.dma_start(
                        out=dmkap(xpad_loc[:], (b * 10 + d + 1) * PLANE + 58 + 1,
                                  [[B * 10 * PLANE, CPG], [58, H], [1, W]]),
                        in_=dmkap(xs32_d[:], b * V + d * H * W,
                                  [[BV, CPG], [W, H], [1, W]]))
                    nc.sync.dma_start(
                        out=dmkap(xw_loc[:], b * XVOL + (d + 3) * XPL + 3 * 62 + 3,
                                  [[B * XVOL, CPG], [62, H], [1, W]]),
                        in_=dmkap(xs32_d[:], b * V + d * H * W,
                                  [[BV, CPG], [W, H], [1, W]]))
            tc.strict_bb_all_engine_barrier()
            nc.gpsimd.collective_compute(
                "AllGather", ALU.bypass, replica_groups=RG,
                ins=[xpad_loc[:].opt()], outs=[xpad_full[:].opt()])
            tc.strict_bb_all_engine_barrier()
            conv_phase(nc, tc, xpad_full, wt1_sb, bo1_sb, off1_d, "1")
            tc.strict_bb_all_engine_barrier()
            dense_phase(nc, tc, xw_loc, off1_d, wd1_sb, part1_d, cols1_d, "1")
            tc.strict_bb_all_engine_barrier()
            nc.gpsimd.collective_compute(
                "ReduceScatter", ALU.add, replica_groups=RG,
                ins=[part1_d[:].opt()], outs=[h1s_d[:].opt()])
            tc.strict_bb_all_engine_barrier()
            bn1_phase(nc, tc, h1s_d, gb1_sb, hpad_loc, hw_loc)
            tc.strict_bb_all_engine_barrier()
            nc.gpsimd.collective_compute(
                "AllGather", ALU.bypass, replica_groups=RG,
                ins=[hpad_loc[:].opt()], outs=[hpad_full[:].opt()])
            tc.strict_bb_all_engine_barrier()
            conv_phase(nc, tc, hpad_full, wt2_sb, bo2_sb, off2_d, "2")
            tc.strict_bb_all_engine_barrier()
            dense_phase(nc, tc, hw_loc, off2_d, wd2_sb, part2_d, cols2_d, "2")
            tc.strict_bb_all_engine_barrier()
            nc.gpsimd.collective_compute(
                "ReduceScatter", ALU.add, replica_groups=RG,
                ins=[part2_d[:].opt()], outs=[h2s_d[:].opt()])
            tc.strict_bb_all_engine_barrier()
            bn2_phase(nc, tc, h2s_d, gb2_sb, xs32_d, out_par)
            if DEBUG_TAPS:
                tc.strict_bb_all_engine_barrier()
                taps = [
                    ("dbg_xs", xs32_d, CPG, BV),
                    ("dbg_xpad", xpad_loc, CPG, B * 10 * PLANE),
                    ("dbg_ag", xpad_full, 64, B * 10 * PLANE),
                    ("dbg_off", off1_d, OCG, B * D * 3136),
                    ("dbg_cols", cols1_d, P, B * K * CH),
                    ("dbg_part", part1_d, 64, BV),
                    ("dbg_h1s", h1s_d, CPG, BV),
                    ("dbg_hpad", hpad_loc, CPG, B * 10 * PLANE),
                ]
                TAPN = 2048
                for nm, src, nparts, rowsz in taps:
                    dbg = nc.declare_dram_parameter(nm, [nparts, TAPN], F32,
                                                    isOutput=True)
                    nc.sync.dma_start(
                        out=dbg[:],
                        in_=dmkap(src[:], 0, [[rowsz, nparts], [1, TAPN]]))
    return nc


# ------------------------------------------------------------- the runner --
_RUNNER = None


def _get_runner():
    global _RUNNER
    if _RUNNER is None:
        import jax
        from jax.sharding import Mesh, PartitionSpec
        from jax.experimental.shard_map import shard_map

        _b2j.install_neuronx_cc_hook()
        nc = _build_program()
        partition_name = (nc.partition_id_tensor.name
                          if nc.partition_id_tensor else None)
        in_names, out_names, out_avals, out_shapes = [], [], [], []
        for alloc in nc.m.functions[0].allocations:
            if not isinstance(alloc, mybir.MemoryLocationSet):
                continue
            name = alloc.memorylocations[0].name
            if alloc.kind == "ExternalInput":
                if name != partition_name:
                    in_names.append(name)
            elif alloc.kind == "ExternalOutput":
                out_names.append(name)
                shape = tuple(alloc.tensor_shape)
                dt = mybir.dt.np(alloc.dtype)
                out_avals.append(jax.core.ShapedArray(shape, dt))
                out_shapes.append((shape, dt))
        n_params = len(in_names)
        n_outs = len(out_names)
        # "out" aliases the donated "xs" buffer (same per-core shape+dtype;
        # xs is fully consumed in the widen phase before out is written), so
        # no zero-filled output ride-alongs need to cross the tunnel.
        all_in = list(in_names)
        if partition_name is not None:
            all_in.append(partition_name)
        donate = (in_names.index("xs"),)

        def _body(*args):
            operands = list(args)
            if partition_name is not None:
                operands.append(_b2j.partition_id_tensor())
            outs = _b2j._bass_exec_p.bind(
                *operands, out_avals=tuple(out_avals), in_names=tuple(all_in),
                out_names=tuple(out_names), lowering_input_output_aliases=(),
                sim_require_finite=True, sim_require_nnan=True, nc=nc)
            return tuple(outs)

        devices = jax.devices()[:NCORES]
        mesh = Mesh(np.asarray(devices), ("core",))
        from jax.sharding import NamedSharding
        sharding = NamedSharding(mesh, PartitionSpec("core"))
        sharded = jax.jit(shard_map(
            _body, mesh=mesh,
            in_specs=(PartitionSpec("core"),) * n_params,
            out_specs=(PartitionSpec("core"),) * n_outs,
            check_rep=False), donate_argnums=donate, keep_unused=True)
        _RUNNER = (sharded, in_names, out_names, out_shapes, sharding)
    return _RUNNER


_DEVW = {}


def _weight_feeds(inputs, sharding):
    """Prep + device-put the weight/BN feeds, cached across calls (weights
    stay resident on device; x is still uploaded and recomputed each call)."""
    import jax
    import ml_dtypes
    names = ("w_off1", "b_off1", "w_dc1", "w_off2", "b_off2", "w_dc2",
             "gamma1", "beta1", "gamma2", "beta2")
    arrs = [np.asarray(inputs[n]) for n in names]
    key = tuple((id(a), a.shape) for a in arrs) + tuple(
        a.ravel()[:256].tobytes() for a in arrs)
    hit = _DEVW.get("key") == key
    if not hit:
        def prep_layer(w_off, b_off, w_dc):
            w_off = np.asarray(w_off, np.float32).reshape(G, OCG, 64, K)
            wt = np.ascontiguousarray(w_off.transpose(0, 2, 3, 1)).reshape(
                G * 64, K * OCG).astype(ml_dtypes.bfloat16)
            bo = np.ascontiguousarray(
                np.asarray(b_off, np.float32)).reshape(G * OCG, 1)
            w_dc = np.asarray(w_dc, np.float32).reshape(64, G, CPG, K)
            wd = np.ascontiguousarray(w_dc.transpose(1, 2, 3, 0)).reshape(
                G * CPG, K * 64).astype(ml_dtypes.bfloat16)
            return wt, bo, wd

        wt1, bo1, wd1 = prep_layer(inputs["w_off1"], inputs["b_off1"],
                                   inputs["w_dc1"])
        wt2, bo2, wd2 = prep_layer(inputs["w_off2"], inputs["b_off2"],
                                   inputs["w_dc2"])
        gb1 = np.ascontiguousarray(np.stack(
            [np.asarray(inputs["gamma1"], np.float32),
             np.asarray(inputs["beta1"], np.float32)], axis=1))
        gb2 = np.ascontiguousarray(np.stack(
            [np.asarray(inputs["gamma2"], np.float32),
             np.asarray(inputs["beta2"], np.float32)], axis=1))
        host = {"wt1": wt1, "bo1": bo1, "wd1": wd1,
                "wt2": wt2, "bo2": bo2, "wd2": wd2, "gb1": gb1, "gb2": gb2}
        dev = {k: jax.device_put(v, sharding) for k, v in host.items()}
        jax.block_until_ready(list(dev.values()))
        _DEVW.clear()
        _DEVW.update({"key": key, "dev": dev})
    return _DEVW["dev"]


def kernel(**inputs):
    import ml_dtypes
    x = np.ascontiguousarray(
        np.asarray(inputs["x"], np.float32).transpose(1, 0, 2, 3, 4)
        .reshape(64, BV)).astype(ml_dtypes.bfloat16)

    sharded, in_names, out_names, out_shapes, sharding = _get_runner()
    feeds = dict(_weight_feeds(inputs, sharding))
    feeds["xs"] = x
    out_arrs = sharded(*[feeds[n] for n in in_names])
    out = np.asarray(out_arrs[out_names.index("out")]).astype(np.float32)
    return np.ascontiguousarray(
        out.reshape(64, B, D, H, W).transpose(1, 0, 2, 3, 4))


# revision 5
# speedup vs baseline: 1.0577x; 1.0577x over previous
"""Trainium2 Bass kernel for nn_DeformBasicBlock1 (deformable conv block).

Single SPMD invocation over 8 cores, sharded by channel-group g (8
x-channels / 81 offset channels / 8 output channels per core).  All
cross-core exchange happens on-device: AllGather replicates the padded
activation volumes for the offset convs, ReduceScatter sums the per-group
deform-conv partials so each core lands exactly its own 8 output channels
for the per-channel BN.  Host traffic per call is just the sharded inputs
(~23 MB) and the sharded output (~13 MB); the compiled executable is cached
across calls.

Deform sampling follows the proven dense 5x5x5 shifted-hat expansion
(offsets clamped to +/-1.999), contracted with the deform weights on the PE.
"""
import json
import os
import numpy as np

os.environ.setdefault("JAX_COMPILATION_CACHE_DIR", "/tmp/jax_cache_deform")
os.environ.setdefault("JAX_PERSISTENT_CACHE_MIN_COMPILE_TIME_SECS", "1")

import concourse.bass as bass
import concourse.mybir as mybir
from concourse.tile import TileContext
import concourse.bass_utils as bass_utils
import concourse.tile_utils as tile_utils

# ---------------------------------------------------------------- tilefix --
# This container's walrus rejects >1 sem-wait per instruction; split extra
# waits onto preceding same-engine NoOps (program order preserves wait
# semantics).
_orig_compile_bir_kernel = bass_utils.compile_bir_kernel


def _split_waits_json(bir_json: bytes) -> bytes:
    j = json.loads(bir_json)
    ctr = 0
    changed = False
    for f in j["functions"]:
        for b in f["blocks"]:
            insts = b["instructions"]
            if not any(
                len((i.get("sync_info") or {}).get("on_wait") or []) > 1
                for i in insts
            ):
                continue
            changed = True
            out = []
            for inst in insts:
                si = inst.get("sync_info")
                if si:
                    ow = si.get("on_wait") or []
                    if len(ow) > 1:
                        for w in ow[:-1]:
                            ctr += 1
                            nop = {
                                "engine": inst["engine"],
                                "ins": [],
                                "outs": [],
                                "name": f"WSPLIT-{ctr}",
                                "opcode": "NoOp",
                                "sync_info": {"on_update": [], "on_wait": [w]},
                            }
                            if "debug" in inst:
                                nop["debug"] = inst["debug"]
                            out.append(nop)
                        si["on_wait"] = [ow[-1]]
                out.append(inst)
            b["instructions"] = out
    return json.dumps(j).encode() if changed else bir_json


def _patched_compile_bir_kernel(bir_json, tmpdir, neff_name="file.neff"):
    if isinstance(bir_json, str):
        bir_json = bir_json.encode()
    return _orig_compile_bir_kernel(_split_waits_json(bir_json), tmpdir, neff_name)


bass_utils.compile_bir_kernel = _patched_compile_bir_kernel
import concourse.bass2jax as _b2j  # noqa: E402

_b2j.compile_bir_kernel = _patched_compile_bir_kernel
try:
    tile_utils.max_sbuf_usage = 204 * 1024
except Exception:
    pass

# ------------------------------------------------------------- constants --
NCORES = 8
B, D, H, W = 2, 8, 56, 56
CPG, G, K = 8, 8, 27
OCG = 81
V = D * H * W
BV = B * V
PLANE = 3364  # 58*58
NB, BH = 14, 4
P = NB * CPG  # 112
CH = D * BH * W  # 1792
XD, XH, XWW = 14, 10, 62
XSZ = XD * XH * XWW
XVOL = XD * 62 * 62
XPL = 62 * 62  # 3844
SS = 5
CLAMP = 1.999
F32 = mybir.dt.float32
BF16 = mybir.dt.bfloat16
AX = mybir.AxisListType
ALU = mybir.AluOpType
ACTF = mybir.ActivationFunctionType
RG = [list(range(NCORES))]


def mkap(tile, off, dims):
    ap = tile[:]
    return bass.AP(tensor=ap.tensor, offset=ap.offset + off,
                   ap=[list(ap.ap[0])] + [list(d) for d in dims])


def dmkap(t_ap, off, dims):
    return bass.AP(tensor=t_ap.tensor, offset=t_ap.offset + off,
                   ap=[list(d) for d in dims])


ZCHUNK = 4096


def zero_dram(nc, zero_sb, dram_ap, nparts, total):
    for off in range(0, total, ZCHUNK):
        sz = min(ZCHUNK, total - off)
        nc.sync.dma_start(
            out=dmkap(dram_ap, off, [[total, nparts], [1, sz]]),
            in_=dmkap(zero_sb[:], 0, [[ZCHUNK, nparts], [1, sz]]))


def conv_phase(nc, tc, xpad_dram, wt_sb, bias_sb, off_dram, sfx):
    """27-tap conv: xpad_dram [64, B*10*PLANE] -> off_dram [81, B*D*3136]."""
    GUARD = 64
    # row-aligned n-chunks of the padded plane: (n0, nsz, valid_row_start, n_valid)
    CHUNKS = []
    for r0 in range(0, 58, 8):
        nr = min(8, 58 - r0)
        v0 = max(1, r0)
        v1 = min(57, r0 + nr)
        CHUNKS.append((r0 * 58, nr * 58, v0 - r0, v1 - v0))
    with tc.tile_pool(name=f"convp{sfx}", bufs=2) as pool, \
         tc.tile_pool(name=f"convps{sfx}", bufs=4, space="PSUM") as pspool:
        for b in range(B):
            for j in range(4):
                xpc = pool.tile([64, 2 * GUARD + 4 * PLANE], F32, tag="xpc")
                nc.vector.memset(xpc[:, :GUARD], 0.0)
                nc.vector.memset(xpc[:, GUARD + 4 * PLANE:], 0.0)
                nc.sync.dma_start(
                    out=xpc[:, GUARD:GUARD + 4 * PLANE],
                    in_=dmkap(xpad_dram[:], (b * 10 + 2 * j) * PLANE,
                              [[B * 10 * PLANE, 64], [1, 4 * PLANE]]))
                for ds in range(2):
                    d = 2 * j + ds
                    for (n0, nsz, vr, nv) in CHUNKS:
                        ps = pspool.tile([OCG, 512], F32, tag="cps")
                        for k in range(K):
                            kd, kh, kw = k // 9, (k // 3) % 3, k % 3
                            roff = GUARD + (ds + kd) * PLANE + (kh - 1) * 58 + (kw - 1) + n0
                            nc.tensor.matmul(ps[:, :nsz], wt_sb[:, k, :],
                                             mkap(xpc, roff, [[1, nsz]]),
                                             start=(k == 0), stop=(k == K - 1))
                        ot = pool.tile([OCG, 512], F32, tag="convot")
                        nc.vector.tensor_tensor(
                            out=ot[:, :nsz], in0=ps[:, :nsz],
                            in1=mkap(bias_sb, 0, [[0, nsz]]), op=ALU.add)
                        if nv <= 0:
                            continue
                        # store valid interior rows to canonical [81, B, D, 56, 56]
                        real_r0 = n0 // 58 + vr - 1
                        nc.sync.dma_start(
                            out=dmkap(off_dram[:], (b * D + d) * 3136 + real_r0 * 56,
                                      [[B * D * 3136, OCG], [1, nv * 56]]),
                            in_=mkap(ot, vr * 58 + 1, [[58, nv], [1, 56]]))


def dense_phase(nc, tc, xw_dram, off_dram, wd_sb, partial_dram, colsd_dram, sfx):
    """Dense 5^3 deform + einsum -> partial_dram [64, B*V] (band-perm)."""
    with tc.tile_pool(name=f"densep{sfx}", bufs=1) as pool, \
         tc.tile_pool(name=f"densew{sfx}", bufs=1) as wpool, \
         tc.tile_pool(name=f"denseps{sfx}", bufs=2, space="PSUM") as pspool:
        for b in range(B):
            xw = pool.tile([P, XSZ], F32, tag="xw")
            for dd in range(XD):
                nc.sync.dma_start(
                    out=mkap(xw, dd * XH * XWW, [[1, 620]]),
                    in_=dmkap(xw_dram[:], b * XVOL + dd * 62 * 62,
                              [[BH * XWW, NB], [B * XVOL, CPG], [1, XH * XWW]]))
            for k in range(K):
                kd, kh, kw = k // 9 - 1, (k // 3) % 3 - 1, k % 3 - 1
                offt = pool.tile([P, 3, CH], F32, tag="offt")
                for ax in range(3):
                    for dd in range(D):
                        nc.sync.dma_start(
                            out=mkap(offt, ax * CH + dd * BH * W, [[1, BH * W]]),
                            in_=dmkap(off_dram[:],
                                      (3 * k + ax) * B * D * 3136 + (b * D + dd) * 3136,
                                      [[BH * W, NB], [0, CPG], [1, BH * W]]))
                nc.vector.tensor_scalar(out=offt[:], in0=offt[:], scalar1=CLAMP,
                                        scalar2=-CLAMP, op0=ALU.min, op1=ALU.max)
                hw = pool.tile([P, SS, CH], F32, tag="hw")
                for a in range(SS):
                    nc.scalar.activation(hw[:, a, :], offt[:, 2, :], ACTF.Abs,
                                         bias=float(-(a - 2)), scale=1.0)
                    nc.scalar.activation(hw[:, a, :], hw[:, a, :], ACTF.Relu,
                                         bias=1.0, scale=-1.0)
                cols = wpool.tile([P, CH], F32, tag="cols")
                pt = wpool.tile([P, CH], F32, tag="pt")
                at = wpool.tile([P, CH], F32, tag="at")
                tt = wpool.tile([P, CH], F32, tag="tt")
                hdsl = pool.tile([P, CH], F32, tag="hdsl")
                hhsl = pool.tile([P, CH], F32, tag="hhsl")
                first = True
                for sd in range(SS):
                    nc.scalar.activation(hdsl[:], offt[:, 0, :], ACTF.Abs,
                                         bias=float(-(sd - 2)), scale=1.0)
                    nc.scalar.activation(hdsl[:], hdsl[:], ACTF.Relu,
                                         bias=1.0, scale=-1.0)
                    for sh in range(SS):
                        nc.scalar.activation(hhsl[:], offt[:, 1, :], ACTF.Abs,
                                             bias=float(-(sh - 2)), scale=1.0)
                        nc.scalar.activation(hhsl[:], hhsl[:], ACTF.Relu,
                                             bias=1.0, scale=-1.0)
                        nc.vector.tensor_tensor(out=pt[:], in0=hdsl[:],
                                                in1=hhsl[:], op=ALU.mult)
                        for sw in range(SS):
                            xoff = ((1 + kd + sd) * XH * XWW + (1 + kh + sh) * XWW
                                    + (1 + kw + sw))
                            xap = mkap(xw, xoff, [[XH * XWW, D], [XWW, BH], [1, W]])
                            dst = at if sw == 0 else tt
                            nc.vector.tensor_tensor(out=dst[:], in0=xap,
                                                    in1=hw[:, sw, :], op=ALU.mult)
                            if sw > 0:
                                nc.vector.tensor_tensor(out=at[:], in0=at[:],
                                                        in1=tt[:], op=ALU.add)
                        if first:
                            nc.vector.tensor_tensor(out=cols[:], in0=pt[:], in1=at[:],
                                                    op=ALU.mult)
                            first = False
                        else:
                            nc.gpsimd.tensor_tensor(out=tt[:], in0=pt[:], in1=at[:],
                                                    op=ALU.mult)
                            nc.gpsimd.tensor_tensor(out=cols[:], in0=cols[:], in1=tt[:],
                                                    op=ALU.add)
                nc.sync.dma_start(
                    out=dmkap(colsd_dram[:], (b * K + k) * CH,
                              [[B * K * CH, P], [1, CH]]),
                    in_=cols[:])
            tc.strict_bb_all_engine_barrier()
            # einsum: psum accumulate over taps per band
            for hb in range(NB):
                ps2 = pspool.tile([64, 2048], F32, tag="eps")
                for k in range(K):
                    cr = wpool.tile([CPG, CH], F32, tag="colsr")
                    nc.sync.dma_start(
                        out=cr[:],
                        in_=dmkap(colsd_dram[:], hb * CPG * B * K * CH + (b * K + k) * CH,
                                  [[B * K * CH, CPG], [1, CH]]))
                    for i in range(4):
                        nc.tensor.matmul(ps2[:, i * 512:i * 512 + 448], wd_sb[:, k, :],
                                         cr[:, i * 448:(i + 1) * 448],
                                         start=(k == 0), stop=(k == K - 1))
                pot = wpool.tile([64, CH], F32, tag="pot")
                nc.vector.tensor_copy(out=pot[:], in_=mkap(ps2, 0, [[512, 4], [1, 448]]))
                nc.sync.dma_start(
                    out=dmkap(partial_dram[:], b * V + hb * CH, [[B * V, 64], [1, CH]]),
                    in_=pot[:])


def ensure_consts(nc):
    for v in (2.0, -2.0, -1.0, 1e-5):
        key = (F32, v)
        if key not in nc.const_aps.aps:
            t = nc.alloc_sbuf_tensor(f"const-f32-{v}", [128, 1], F32)
            nc.gpsimd.memset(t.ap(), v)
            nc.const_aps.aps[key] = t.ap()


def bn_stats_g(nc, pool, spool, src_dram, gamma_ap, beta_ap, sfx):
    """Per-channel BN scale/shift for the CPG-channel shard src [CPG, B*V]."""
    NCHK = 16
    CSZ = BV // NCHK
    sum_t = spool.tile([CPG, 1], F32, tag=f"bsum{sfx}")
    sq_t = spool.tile([CPG, 1], F32, tag=f"bsq{sfx}")
    nc.vector.memset(sum_t[:], 0.0)
    nc.vector.memset(sq_t[:], 0.0)
    for i in range(NCHK):
        ht = pool.tile([CPG, CSZ], F32, tag="bst")
        sqv = pool.tile([CPG, CSZ], F32, tag="bsv")
        t1 = pool.tile([CPG, 1], F32, tag="bt1")
        t2 = pool.tile([CPG, 1], F32, tag="bt2")
        nc.sync.dma_start(out=ht[:],
                          in_=dmkap(src_dram[:], i * CSZ, [[BV, CPG], [1, CSZ]]))
        nc.vector.tensor_reduce(out=t1[:], in_=ht[:], axis=AX.X, op=ALU.add)
        nc.vector.tensor_tensor(out=sqv[:], in0=ht[:], in1=ht[:], op=ALU.mult)
        nc.vector.tensor_reduce(out=t2[:], in_=sqv[:], axis=AX.X, op=ALU.add)
        nc.vector.tensor_tensor(out=sum_t[:], in0=sum_t[:], in1=t1[:], op=ALU.add)
        nc.vector.tensor_tensor(out=sq_t[:], in0=sq_t[:], in1=t2[:], op=ALU.add)
    N = float(BV)
    scale = spool.tile([CPG, 1], F32, tag=f"bscale{sfx}")
    shift = spool.tile([CPG, 1], F32, tag=f"bshift{sfx}")
    mean = spool.tile([CPG, 1], F32, tag=f"bmean{sfx}")
    var = spool.tile([CPG, 1], F32, tag=f"bvar{sfx}")
    msq = spool.tile([CPG, 1], F32, tag=f"bmsq{sfx}")
    rstd = spool.tile([CPG, 1], F32, tag=f"brstd{sfx}")
    nc.vector.tensor_scalar(out=mean[:], in0=sum_t[:], scalar1=1.0 / N, scalar2=0.0,
                            op0=ALU.mult, op1=ALU.add)
    nc.vector.tensor_scalar(out=var[:], in0=sq_t[:], scalar1=1.0 / N, scalar2=0.0,
                            op0=ALU.mult, op1=ALU.add)
    nc.vector.tensor_tensor(out=msq[:], in0=mean[:], in1=mean[:], op=ALU.mult)
    nc.vector.tensor_tensor(out=var[:], in0=var[:], in1=msq[:], op=ALU.subtract)
    nc.scalar.activation(out=rstd[:], in_=var[:], func=ACTF.Sqrt, bias=1e-5, scale=1.0)
    nc.vector.reciprocal(out=rstd[:], in_=rstd[:])
    nc.vector.tensor_tensor(out=scale[:], in0=gamma_ap, in1=rstd[:], op=ALU.mult)
    nc.vector.tensor_tensor(out=shift[:], in0=mean[:], in1=scale[:], op=ALU.mult)
    nc.vector.tensor_tensor(out=shift[:], in0=beta_ap, in1=shift[:], op=ALU.subtract)
    return scale, shift


def bn1_phase(nc, tc, hs_dram, gb_sb, hpad_loc, hw_loc):
    """BN+ReLU on the 8-channel shard (band-perm) and scatter into the padded
    DRAM layouts for layer 2."""
    with tc.tile_pool(name="bn1p", bufs=2) as pool, \
         tc.tile_pool(name="bn1s", bufs=1) as spool:
        scale, shift = bn_stats_g(nc, pool, spool, hs_dram,
                                  gb_sb[:, 0:1], gb_sb[:, 1:2], "1")
        for b in range(B):
            for hb in range(NB):
                ht = pool.tile([CPG, CH], F32, tag="bna")
                nc.sync.dma_start(
                    out=ht[:],
                    in_=dmkap(hs_dram[:], (b * NB + hb) * CH, [[BV, CPG], [1, CH]]))
                nc.scalar.activation(out=ht[:], in_=ht[:], func=ACTF.Relu,
                                     bias=shift[:], scale=scale[:])
                for d in range(D):
                    src = mkap(ht, d * BH * W, [[W, BH], [1, W]])
                    nc.sync.dma_start(
                        out=dmkap(hpad_loc[:],
                                  (b * 10 + d + 1) * PLANE + (hb * BH + 1) * 58 + 1,
                                  [[B * 10 * PLANE, CPG], [58, BH], [1, W]]),
                        in_=src)
                    src2 = mkap(ht, d * BH * W, [[W, BH], [1, W]])
                    nc.sync.dma_start(
                        out=dmkap(hw_loc[:],
                                  b * XVOL + (d + 3) * XPL + (hb * BH + 3) * 62 + 3,
                                  [[B * XVOL, CPG], [62, BH], [1, W]]),
                        in_=src2)


def bn2_phase(nc, tc, hs_dram, gb_sb, xs_in, out_par):
    """BN + residual + ReLU on the 8-channel shard; write canonical output."""
    with tc.tile_pool(name="bn2p", bufs=2) as pool, \
         tc.tile_pool(name="bn2s", bufs=1) as spool:
        scale, shift = bn_stats_g(nc, pool, spool, hs_dram,
                                  gb_sb[:, 0:1], gb_sb[:, 1:2], "2")
        for b in range(B):
            for hb in range(NB):
                ht = pool.tile([CPG, CH], F32, tag="b2h")
                rt = pool.tile([CPG, CH], F32, tag="b2r")
                nc.sync.dma_start(
                    out=ht[:],
                    in_=dmkap(hs_dram[:], (b * NB + hb) * CH, [[BV, CPG], [1, CH]]))
                nc.sync.dma_start(
                    out=rt[:],
                    in_=dmkap(xs_in[:], b * V + hb * BH * W,
                              [[BV, CPG], [H * W, D], [1, BH * W]]))
                nc.vector.tensor_tensor(out=ht[:], in0=ht[:],
                                        in1=mkap(scale, 0, [[0, CH]]), op=ALU.mult)
                nc.vector.tensor_tensor(out=ht[:], in0=ht[:],
                                        in1=mkap(shift, 0, [[0, CH]]), op=ALU.add)
                nc.vector.tensor_tensor(out=ht[:], in0=ht[:], in1=rt[:], op=ALU.add)
                ob = pool.tile([CPG, CH], BF16, tag="b2o")
                nc.vector.tensor_scalar(out=ob[:], in0=ht[:], scalar1=0.0,
                                        scalar2=0.0, op0=ALU.max, op1=ALU.add)
                nc.sync.dma_start(
                    out=dmkap(out_par[:], b * V + hb * BH * W,
                              [[BV, CPG], [H * W, D], [1, BH * W]]),
                    in_=ob[:])


DEBUG_TAPS = bool(os.environ.get("KERNEL_DEBUG_TAPS"))


def _build_program():
    nc = bass.Bass("TRN2", target_bir_lowering=False, num_devices=NCORES)
    ensure_consts(nc)
    xs_in = nc.declare_dram_parameter("xs", [CPG, BV], BF16, isOutput=False)
    wt1_in = nc.declare_dram_parameter("wt1", [64, K * OCG], BF16, isOutput=False)
    bo1_in = nc.declare_dram_parameter("bo1", [OCG, 1], F32, isOutput=False)
    wd1_in = nc.declare_dram_parameter("wd1", [CPG, K * 64], BF16, isOutput=False)
    wt2_in = nc.declare_dram_parameter("wt2", [64, K * OCG], BF16, isOutput=False)
    bo2_in = nc.declare_dram_parameter("bo2", [OCG, 1], F32, isOutput=False)
    wd2_in = nc.declare_dram_parameter("wd2", [CPG, K * 64], BF16, isOutput=False)
    gb1_in = nc.declare_dram_parameter("gb1", [CPG, 2], F32, isOutput=False)
    gb2_in = nc.declare_dram_parameter("gb2", [CPG, 2], F32, isOutput=False)
    out_par = nc.declare_dram_parameter("out", [CPG, BV], BF16, isOutput=True)

    xs32_d = nc.dram_tensor("xs32_d", [CPG, BV], F32)
    xpad_loc = nc.dram_tensor("xpad_loc", [CPG, B * 10 * PLANE], F32)
    xpad_full = nc.dram_tensor("xpad_full", [64, B * 10 * PLANE], F32)
    xw_loc = nc.dram_tensor("xw_loc", [CPG, B * XVOL], F32)
    off1_d = nc.dram_tensor("off1_d", [OCG, B * D * 3136], F32)
    cols1_d = nc.dram_tensor("cols1_d", [P, B * K * CH], F32)
    part1_d = nc.dram_tensor("part1_d", [64, BV], F32)
    h1s_d = nc.dram_tensor("h1s_d", [CPG, BV], F32)
    hpad_loc = nc.dram_tensor("hpad_loc", [CPG, B * 10 * PLANE], F32)
    hpad_full = nc.dram_tensor("hpad_full", [64, B * 10 * PLANE], F32)
    hw_loc = nc.dram_tensor("hw_loc", [CPG, B * XVOL], F32)
    off2_d = nc.dram_tensor("off2_d", [OCG, B * D * 3136], F32)
    cols2_d = nc.dram_tensor("cols2_d", [P, B * K * CH], F32)
    part2_d = nc.dram_tensor("part2_d", [64, BV], F32)
    h2s_d = nc.dram_tensor("h2s_d", [CPG, BV], F32)

    with TileContext(nc) as tc:
        with tc.tile_pool(name="persist", bufs=1) as sp:
            zero_sb = sp.tile([CPG, ZCHUNK], F32, tag="zsb")
            nc.vector.memset(zero_sb[:], 0.0)
            wt1_sb = sp.tile([64, K, OCG], F32, tag="wt1")
            bo1_sb = sp.tile([OCG, 1], F32, tag="bo1")
            nc.sync.dma_start(out=bo1_sb[:], in_=bo1_in[:])
            wd1_sb = sp.tile([CPG, K, 64], F32, tag="wd1")
            wt2_sb = sp.tile([64, K, OCG], F32, tag="wt2")
            bo2_sb = sp.tile([OCG, 1], F32, tag="bo2")
            nc.sync.dma_start(out=bo2_sb[:], in_=bo2_in[:])
            wd2_sb = sp.tile([CPG, K, 64], F32, tag="wd2")
            with tc.tile_pool(name="wload", bufs=1) as wl:
                for src, dst, parts, cols in ((wt1_in, wt1_sb, 64, K * OCG),
                                              (wt2_in, wt2_sb, 64, K * OCG),
                                              (wd1_in, wd1_sb, CPG, K * 64),
                                              (wd2_in, wd2_sb, CPG, K * 64)):
                    stg = wl.tile([parts, cols], BF16, tag=f"stg{parts}")
                    nc.sync.dma_start(out=stg[:], in_=src[:])
                    nc.vector.tensor_copy(out=dst[:], in_=stg[:])
            gb1_sb = sp.tile([CPG, 2], F32, tag="gb1")
            nc.sync.dma_start(out=gb1_sb[:], in_=gb1_in[:])
            gb2_sb = sp.tile([CPG, 2], F32, tag="gb2")
            nc.sync.dma_start(out=gb2_sb[:], in_=gb2_in[:])

            # zero the padded scratch volumes (borders stay zero throughout)
            zero_dram(nc, zero_sb, xpad_loc[:], CPG, B * 10 * PLANE)
            zero_dram(nc, zero_sb, xw_loc[:], CPG, B * XVOL)
            zero_dram(nc, zero_sb, hpad_loc[:], CPG, B * 10 * PLANE)
            zero_dram(nc, zero_sb, hw_loc[:], CPG, B * XVOL)
            # widen the bf16 input shard to f32
            with tc.tile_pool(name="cvt", bufs=2) as cpool:
                NCV = 16
                CVS = BV // NCV
                for i in range(NCV):
                    tb = cpool.tile([CPG, CVS], BF16, tag="cvb")
                    tf = cpool.tile([CPG, CVS], F32, tag="cvf")
                    nc.sync.dma_start(
                        out=tb[:],
                        in_=dmkap(xs_in[:], i * CVS, [[BV, CPG], [1, CVS]]))
                    nc.vector.tensor_copy(out=tf[:], in_=tb[:])
                    nc.sync.dma_start(
                        out=dmkap(xs32_d[:], i * CVS, [[BV, CPG], [1, CVS]]),
                        in_=tf[:])
            tc.strict_bb_all_engine_barrier()
            # scatter own x channels into the padded layouts
            for b in range(B):
                for d in range(D):
                    nc.sync.dma_start(
                        out=dmkap(xpad_loc[:], (b * 10 + d + 1) * PLANE + 58 + 1,
                                  [[B * 10 * PLANE, CPG], [58, H], [1, W]]),
                        in_=dmkap(xs32_d[:], b * V + d * H * W,
                                  [[BV, CPG], [W, H], [1, W]]))
                    nc.sync.dma_start(
                        out=dmkap(xw_loc[:], b * XVOL + (d + 3) * XPL + 3 * 62 + 3,
                                  [[B * XVOL, CPG], [62, H], [1, W]]),
                        in_=dmkap(xs32_d[:], b * V + d * H * W,
                                  [[BV, CPG], [W, H], [1, W]]))
            tc.strict_bb_all_engine_barrier()
            nc.gpsimd.collective_compute(
                "AllGather", ALU.bypass, replica_groups=RG,
                ins=[xpad_loc[:].opt()], outs=[xpad_full[:].opt()])
            tc.strict_bb_all_engine_barrier()
            conv_phase(nc, tc, xpad_full, wt1_sb, bo1_sb, off1_d, "1")
            tc.strict_bb_all_engine_barrier()
            dense_phase(nc, tc, xw_loc, off1_d, wd1_sb, part1_d, cols1_d, "1")
            tc.strict_bb_all_engine_barrier()
            nc.gpsimd.collective_compute(
                "ReduceScatter", ALU.add, replica_groups=RG,
                ins=[part1_d[:].opt()], outs=[h1s_d[:].opt()])
            tc.strict_bb_all_engine_barrier()
            bn1_phase(nc, tc, h1s_d, gb1_sb, hpad_loc, hw_loc)
            tc.strict_bb_all_engine_barrier()
            nc.gpsimd.collective_compute(
                "AllGather", ALU.bypass, replica_groups=RG,
                ins=[hpad_loc[:].opt()], outs=[hpad_full[:].opt()])
            tc.strict_bb_all_engine_barrier()
            conv_phase(nc, tc, hpad_full, wt2_sb, bo2_sb, off2_d, "2")
            tc.strict_bb_all_engine_barrier()
            dense_phase(nc, tc, hw_loc, off2_d, wd2_sb, part2_d, cols2_d, "2")
            tc.strict_bb_all_engine_barrier()
            nc.gpsimd.collective_compute(
                "ReduceScatter", ALU.add, replica_groups=RG,
                ins=[part2_d[:].opt()], outs=[h2s_d[:].opt()])
            tc.strict_bb_all_engine_barrier()
            bn2_phase(nc, tc, h2s_d, gb2_sb, xs32_d, out_par)
            if DEBUG_TAPS:
                tc.strict_bb_all_engine_barrier()
                taps = [
                    ("dbg_xs", xs32_d, CPG, BV),
                    ("dbg_xpad", xpad_loc, CPG, B * 10 * PLANE),
                    ("dbg_ag", xpad_full, 64, B * 10 * PLANE),
                    ("dbg_off", off1_d, OCG, B * D * 3136),
                    ("dbg_cols", cols1_d, P, B * K * CH),
                    ("dbg_part", part1_d, 64, BV),
                    ("dbg_h1s", h1s_d, CPG, BV),
                    ("dbg_hpad", hpad_loc, CPG, B * 10 * PLANE),
                ]
                TAPN = 2048
                for nm, src, nparts, rowsz in taps:
                    dbg = nc.declare_dram_parameter(nm, [nparts, TAPN], F32,
                                                    isOutput=True)
                    nc.sync.dma_start(
                        out=dbg[:],
                        in_=dmkap(src[:], 0, [[rowsz, nparts], [1, TAPN]]))
    return nc


# ------------------------------------------------------------- the runner --
_RUNNER = None


def _get_runner():
    global _RUNNER
    if _RUNNER is None:
        import jax
        from jax.sharding import Mesh, PartitionSpec
        from jax.experimental.shard_map import shard_map

        _b2j.install_neuronx_cc_hook()
        nc = _build_program()
        partition_name = (nc.partition_id_tensor.name
                          if nc.partition_id_tensor else None)
        in_names, out_names, out_avals, out_shapes = [], [], [], []
        for alloc in nc.m.functions[0].allocations:
            if not isinstance(alloc, mybir.MemoryLocationSet):
                continue
            name = alloc.memorylocations[0].name
            if alloc.kind == "ExternalInput":
                if name != partition_name:
                    in_names.append(name)
            elif alloc.kind == "ExternalOutput":
                out_names.append(name)
                shape = tuple(alloc.tensor_shape)
                dt = mybir.dt.np(alloc.dtype)
                out_avals.append(jax.core.ShapedArray(shape, dt))
                out_shapes.append((shape, dt))
        n_params = len(in_names)
        n_outs = len(out_names)
        # "out" aliases the donated "xs" buffer (same per-core shape+dtype;
        # xs is fully consumed in the widen phase before out is written), so
        # no zero-filled output ride-alongs need to cross the tunnel.
        all_in = list(in_names)
        if partition_name is not None:
            all_in.append(partition_name)
        donate = (in_names.index("xs"),)

        def _body(*args):
            operands = list(args)
            if partition_name is not None:
                operands.append(_b2j.partition_id_tensor())
            outs = _b2j._bass_exec_p.bind(
                *operands, out_avals=tuple(out_avals), in_names=tuple(all_in),
                out_names=tuple(out_names), lowering_input_output_aliases=(),
                sim_require_finite=True, sim_require_nnan=True, nc=nc)
            return tuple(outs)

        devices = jax.devices()[:NCORES]
        mesh = Mesh(np.asarray(devices), ("core",))
        from jax.sharding import NamedSharding
        sharding = NamedSharding(mesh, PartitionSpec("core"))
        sharded = jax.jit(shard_map(
            _body, mesh=mesh,
            in_specs=(PartitionSpec("core"),) * n_params,
            out_specs=(PartitionSpec("core"),) * n_outs,
            check_rep=False), donate_argnums=donate, keep_unused=True)
        _RUNNER = (sharded, in_names, out_names, out_shapes, sharding)
    return _RUNNER


_DEVW = {}


def _weight_feeds(inputs, sharding):
    """Prep + device-put the weight/BN feeds, cached across calls (weights
    stay resident on device; x is still uploaded and recomputed each call)."""
    import jax
    import ml_dtypes
    names = ("w_off1", "b_off1", "w_dc1", "w_off2", "b_off2", "w_dc2",
             "gamma1", "beta1", "gamma2", "beta2")
    arrs = [np.asarray(inputs[n]) for n in names]
    key = tuple((id(a), a.shape) for a in arrs) + tuple(
        a.ravel()[:256].tobytes() for a in arrs)
    hit = _DEVW.get("key") == key
    if not hit:
        def prep_layer(w_off, b_off, w_dc):
            w_off = np.asarray(w_off, np.float32).reshape(G, OCG, 64, K)
            wt = np.ascontiguousarray(w_off.transpose(0, 2, 3, 1)).reshape(
                G * 64, K * OCG).astype(ml_dtypes.bfloat16)
            bo = np.ascontiguousarray(
                np.asarray(b_off, np.float32)).reshape(G * OCG, 1)
            w_dc = np.asarray(w_dc, np.float32).reshape(64, G, CPG, K)
            wd = np.ascontiguousarray(w_dc.transpose(1, 2, 3, 0)).reshape(
                G * CPG, K * 64).astype(ml_dtypes.bfloat16)
            return wt, bo, wd

        wt1, bo1, wd1 = prep_layer(inputs["w_off1"], inputs["b_off1"],
                                   inputs["w_dc1"])
        wt2, bo2, wd2 = prep_layer(inputs["w_off2"], inputs["b_off2"],
                                   inputs["w_dc2"])
        gb1 = np.ascontiguousarray(np.stack(
            [np.asarray(inputs["gamma1"], np.float32),
             np.asarray(inputs["beta1"], np.float32)], axis=1))
        gb2 = np.ascontiguousarray(np.stack(
            [np.asarray(inputs["gamma2"], np.float32),
             np.asarray(inputs["beta2"], np.float32)], axis=1))
        host = {"wt1": wt1, "bo1": bo1, "wd1": wd1,
                "wt2": wt2, "bo2": bo2, "wd2": wd2, "gb1": gb1, "gb2": gb2}
        dev = {k: jax.device_put(v, sharding) for k, v in host.items()}
        jax.block_until_ready(list(dev.values()))
        _DEVW.clear()
        _DEVW.update({"key": key, "dev": dev})
    return _DEVW["dev"]


def kernel(**inputs):
    import ml_dtypes
    x = np.ascontiguousarray(
        np.asarray(inputs["x"], np.float32).transpose(1, 0, 2, 3, 4)
        .reshape(64, BV)).astype(ml_dtypes.bfloat16)

    sharded, in_names, out_names, out_shapes, sharding = _get_runner()
    feeds = dict(_weight_feeds(inputs, sharding))
    feeds["xs"] = x
    out_arrs = sharded(*[feeds[n] for n in in_names])
    out = np.asarray(out_arrs[out_names.index("out")])
    return out.reshape(64, B, D, H, W).transpose(1, 0, 2, 3, 4).astype(np.float32)
